# revision 1
# baseline (speedup 1.0000x reference)
"""Trainium2 Bass kernel for GNO message passing (nn_GNO_69312182222948).

Strategy (data-parallel over edges, 8 cores):
  - Host gathers per-edge x_sparse[src], x_dense[dst], f_sparse[src] (cheap
    numpy fancy-indexing) and packs them into a partition-dense layout:
    10 edge "streams" x (3 xs + 3 xd) rows = 60 partitions, two column-halves
    at partition bases 0 and 64 -> 120 of 128 partitions per DMA tile.
  - Device: per-edge kernel MLP 3->12->12->3 entirely on TensorE with
    block-diagonal weights replicated 10x (one stream per 12-partition block).
    Layer 1 uses stacked [W1; -W1] so rel = xs - xd is folded into the matmul.
    GELUs on ScalarE (ACT) with per-partition bias, reading PSUM directly.
    Layer-3 outputs k for 4 consecutive tiles packed at 32-partition offsets
    of one [128, T] PSUM tile so the (k + b3) * f_src epilogue runs on DVE at
    full partition width.
  - Device streams msg = (k + b3) * f_src back to DRAM; host does the sorted
    segment mean (np.add.reduceat) and the tiny projection MLP in numpy.
"""

import numpy as np

import concourse.bass as bass
import concourse.mybir as mybir
from concourse.bacc import Bacc
from concourse.tile import TileContext
from concourse.bass_utils import run_bass_kernel_spmd

# Problem sizes (hardcoded per contract)
N_S = 131072
N_D = 131072
E = 8388608
DIM = 3
H = 12

N_CORES = 8
S = 10          # streams per core (10 * 12 = 120 partitions)
T = 1024        # edges per stream per tile
GROUP = 2       # tiles whose k outputs share one [128, T] PSUM tile
NT = 104        # tiles per stream per core (must be % GROUP == 0)
NG = NT // GROUP
L = NT * T                  # edges per stream
E_PC = S * L                # edges per core (padded)
E_PAD = N_CORES * E_PC      # total padded edges

# k/f/msg packed rows: tile m of group at 64m + 3s + j
ROWS_K = np.array([64 * m + 3 * s + j
                   for m in range(GROUP) for s in range(S) for j in range(DIM)])

_BASS_CACHE = {}


def _build_bass():
    if "nc" in _BASS_CACHE:
        return _BASS_CACHE["nc"]
    fp32 = mybir.dt.float32
    nc = Bacc()

    xsxd = nc.dram_tensor("xsxd", [NT // 2, 120, 1024], fp32, kind="ExternalInput")
    fpack = nc.dram_tensor("fpack", [NG, 128, T], fp32, kind="ExternalInput")
    # single packed weight table: cols [0:120]=w1sA, [120:240]=w1sB,
    # [240:360]=w2b, [360:390]=w3b, 390=b1t, 391=b2t, 392=b3t
    wtab = nc.dram_tensor("wtab", [128, 393], fp32, kind="ExternalInput")
    msgout = nc.dram_tensor("msgout", [NG, 128, T], fp32, kind="ExternalOutput")

    GELU = mybir.ActivationFunctionType.Gelu

    with TileContext(nc) as tc:
        with (
            tc.tile_pool(name="wpool", bufs=1) as wpool,
            tc.tile_pool(name="inpool", bufs=4) as inpool,
            tc.tile_pool(name="fpool", bufs=2) as fpool,
            tc.tile_pool(name="g1pool", bufs=2) as g1pool,
            tc.tile_pool(name="g2pool", bufs=2) as g2pool,
            tc.tile_pool(name="kspool", bufs=2) as kspool,
            tc.tile_pool(name="msgpool", bufs=2) as msgpool,
            tc.tile_pool(name="ph1", bufs=2, space="PSUM") as ph1,
            tc.tile_pool(name="ph2", bufs=1, space="PSUM") as ph2,
            tc.tile_pool(name="pk", bufs=1, space="PSUM") as pk,
        ):
            wtab_sb = wpool.tile([128, 393], fp32, tag="wtab")
            nc.sync.dma_start(wtab_sb[:], wtab[:, :])
            w1a_sb = wtab_sb[0:120, 0:120]
            w1b_sb = wtab_sb[0:120, 120:240]
            w2_sb = wtab_sb[0:120, 240:360]
            w3_sb = wtab_sb[0:120, 360:390]
            b1_sb = wtab_sb[0:120, 390:391]
            b2_sb = wtab_sb[0:120, 391:392]
            b3_sb = wtab_sb[:, 392:393]

            for t in range(NT):
                g, m = divmod(t, GROUP)
                if t % 2 == 0:
                    xpair = inpool.tile([120, 1024], fp32, tag="xin")
                    nc.sync.dma_start(xpair[:], xsxd[t // 2, :, :])
                xo = 512 * (t % 2)
                xin = xpair[:, xo:xo + 512]

                if m == 0:
                    f_sb = fpool.tile([128, T], fp32, tag="f")
                    nc.sync.dma_start(f_sb[:], fpack[g, :, :])
                    kb = pk.tile([128, T], fp32, tag="kb")

                h1 = ph1.tile([120, T], fp32, tag="h1")
                nc.tensor.matmul(h1[:, 0:512], w1a_sb, xin,
                                 start=True, stop=True)
                nc.tensor.matmul(h1[:, 512:1024], w1b_sb, xin,
                                 start=True, stop=True)
                h1g = g1pool.tile([120, T], fp32, tag="h1g")
                nc.scalar.activation(h1g[:], h1[:], GELU, bias=b1_sb)

                h2 = ph2.tile([120, T], fp32, tag="h2")
                nc.tensor.matmul(h2[:, 0:512], w2_sb, h1g[:, 0:512],
                                 start=True, stop=True)
                nc.tensor.matmul(h2[:, 512:1024], w2_sb, h1g[:, 512:1024],
                                 start=True, stop=True)
                h2g = g2pool.tile([120, T], fp32, tag="h2g")
                nc.scalar.activation(h2g[:], h2[:], GELU, bias=b2_sb)

                nc.tensor.matmul(kb[64 * m:64 * m + 30, 0:512], w3_sb,
                                 h2g[:, 0:512], start=True, stop=True)
                nc.tensor.matmul(kb[64 * m:64 * m + 30, 512:1024], w3_sb,
                                 h2g[:, 512:1024], start=True, stop=True)

                if m == GROUP - 1:
                    ks = kspool.tile([128, T], fp32, tag="ks")
                    nc.vector.tensor_scalar_add(ks[:], kb[:], b3_sb)
                    msg = msgpool.tile([128, T], fp32, tag="msg")
                    nc.vector.tensor_mul(msg[:], ks[:], f_sb[:])
                    nc.gpsimd.dma_start(msgout[g, :, :], msg[:])

    nc.finalize()
    _BASS_CACHE["nc"] = nc
    return nc


def _erf(x):
    # Abramowitz & Stegun 7.1.26 (|err| <= 1.5e-7), computed in float64
    a1, a2, a3, a4, a5 = (0.254829592, -0.284496736, 1.421413741,
                          -1.453152027, 1.061405429)
    p = 0.3275911
    s = np.sign(x)
    ax = np.abs(x)
    t = 1.0 / (1.0 + p * ax)
    y = 1.0 - (((((a5 * t + a4) * t) + a3) * t + a2) * t + a1) * t * np.exp(-ax * ax)
    return s * y

try:
    from scipy.special import erf as _erf  # noqa: F811 (exact if available)
except Exception:
    pass


def _gelu_np(x):
    return 0.5 * x * (1.0 + _erf(x / np.sqrt(2.0)))


def _pack_inputs(x_sparse, f_sparse, x_dense, W1, b1, W2, b2, W3, b3,
                 edge_src, edge_dst):
    src = np.asarray(edge_src).astype(np.int64)
    dst = np.asarray(edge_dst).astype(np.int64)
    x_sparse = np.asarray(x_sparse, dtype=np.float32)
    f_sparse = np.asarray(f_sparse, dtype=np.float32)
    x_dense = np.asarray(x_dense, dtype=np.float32)

    # Host gather of per-edge features, padded so every core/tile is full.
    XS = np.zeros((E_PAD, DIM), np.float32)
    XD = np.zeros((E_PAD, DIM), np.float32)
    F = np.zeros((E_PAD, DIM), np.float32)  # zero pad => msg contribution 0
    XS[:E] = x_sparse[src]
    XD[:E] = x_dense[dst]
    F[:E] = f_sparse[src]

    # Block-diagonal replicated weights
    W1 = np.asarray(W1, np.float32)
    W2 = np.asarray(W2, np.float32)
    W3 = np.asarray(W3, np.float32)
    wtab = np.zeros((128, 393), np.float32)
    for s in range(S):
        # w1sA contracts rows 0:60 (half 0), w1sB rows 60:120 (half 1)
        wtab[6 * s:6 * s + 3, 12 * s:12 * s + 12] = W1
        wtab[6 * s + 3:6 * s + 6, 12 * s:12 * s + 12] = -W1
        wtab[60 + 6 * s:60 + 6 * s + 3, 120 + 12 * s:120 + 12 * s + 12] = W1
        wtab[60 + 6 * s + 3:60 + 6 * s + 6, 120 + 12 * s:120 + 12 * s + 12] = -W1
        wtab[12 * s:12 * s + 12, 240 + 12 * s:240 + 12 * s + 12] = W2
        wtab[12 * s:12 * s + 12, 360 + 3 * s:360 + 3 * s + 3] = W3
    wtab[0:120, 390] = np.tile(np.asarray(b1, np.float32), S)
    wtab[0:120, 391] = np.tile(np.asarray(b2, np.float32), S)
    wtab[ROWS_K, 392] = np.tile(np.asarray(b3, np.float32), GROUP * S)

    in_maps = []
    for c in range(N_CORES):
        sl = slice(c * E_PC, (c + 1) * E_PC)
        # (S, NT, 2, 512, 3) -> (NT, 2, S, 3, 512)
        xs = XS[sl].reshape(S, NT, 2, 512, DIM).transpose(1, 2, 0, 4, 3)
        xd = XD[sl].reshape(S, NT, 2, 512, DIM).transpose(1, 2, 0, 4, 3)
        # interleave (3 xs + 3 xd) per stream -> row 6s + j within each half
        half = np.concatenate([xs, xd], axis=3).reshape(NT, 2, 60, 512)
        xsxd = half.reshape(NT, 120, 512)  # rows 0:60 half0, 60:120 half1
        # pair consecutive tiles into [120, 1024] DMA tiles
        xsxd = np.concatenate([xsxd[0::2], xsxd[1::2]], axis=2)
        # (S, NG, GROUP, T, 3) -> (NG, GROUP, S, 3, T) -> rows 64m + 3s + j
        fp120 = F[sl].reshape(S, NG, GROUP, T, DIM).transpose(1, 2, 0, 4, 3)
        fp = np.zeros((NG, 128, T), np.float32)
        fp[:, ROWS_K, :] = fp120.reshape(NG, GROUP * S * DIM, T)
        in_maps.append({
            "xsxd": xsxd,
            "fpack": fp,
            "wtab": wtab,
        })
    return in_maps, dst


def _host_tail(outs, dst, P1w, P1b, P2w, P2b, P3w, P3b):
    # Unpack device msg back to global edge order
    msg = np.empty((E_PAD, DIM), np.float32)
    for c in range(N_CORES):
        mo = np.asarray(outs[c]["msgout"])  # [NG, 128, T]
        mo = mo[:, ROWS_K, :].reshape(NG, GROUP, S, DIM, T)
        mo = mo.transpose(2, 0, 1, 4, 3)   # (S, NG, GROUP, T, DIM)
        msg[c * E_PC:(c + 1) * E_PC] = mo.reshape(E_PC, DIM)
    msg = msg[:E]

    # Sorted segment mean
    cnt = np.bincount(dst, minlength=N_D).astype(np.float32)
    starts = (np.cumsum(cnt) - cnt).astype(np.int64)
    nz = cnt > 0
    sums = np.zeros((N_D, DIM), np.float32)
    if nz.any():
        sums[nz] = np.add.reduceat(msg, starts[nz], axis=0)
    out_feat = sums / np.maximum(cnt, 1.0)[:, None]

    # Projection MLP (tiny) on host, float64 internally
    h = _gelu_np(out_feat.astype(np.float64) @ np.asarray(P1w, np.float64)
                 + np.asarray(P1b, np.float64))
    h = _gelu_np(h @ np.asarray(P2w, np.float64) + np.asarray(P2b, np.float64))
    out = h @ np.asarray(P3w, np.float64) + np.asarray(P3b, np.float64)
    return out.astype(np.float32)


def kernel(x_sparse, f_sparse, x_dense, W1, b1, W2, b2, W3, b3,
           P1w, P1b, P2w, P2b, P3w, P3b, edge_src, edge_dst):
    in_maps, dst = _pack_inputs(x_sparse, f_sparse, x_dense, W1, b1, W2, b2,
                                W3, b3, edge_src, edge_dst)
    nc = _build_bass()
    res = run_bass_kernel_spmd(nc, in_maps, list(range(N_CORES)))
    return _host_tail(res.results, dst, P1w, P1b, P2w, P2b, P3w, P3b)


def run_profiled(inputs, tmpdir=None):
    """Run once with tracing enabled; returns BassKernelResults."""
    kw = {k: v for k, v in inputs.items()
          if k in ("x_sparse", "f_sparse", "x_dense", "W1", "b1", "W2", "b2",
                   "W3", "b3", "edge_src", "edge_dst")}
    in_maps, _ = _pack_inputs(**kw)
    nc = _build_bass()
    return run_bass_kernel_spmd(nc, in_maps, list(range(N_CORES)),
                                trace=True, tmpdir=tmpdir)



# revision 8
# speedup vs baseline: 2.1243x; 2.1243x over previous
"""Trainium2 Bass kernel for GNO message passing (nn_GNO_69312182222948).

Strategy (data-parallel over edges, 8 cores):
  - Host gathers per-edge rel = x_sparse[src] - x_dense[dst] (numpy fancy
    indexing) in bf16 and packs it partition-dense: 4 chunks x 30 rows
    (10 streams x 3 dims) at 32-aligned partition bases, 2048 cols per
    input tile -> [128, 2048] bf16 DMA tiles.
  - Device: per-edge kernel MLP 3->12->12->3 on TensorE in bf16 (full
    rate; fp32 runs a 2x LOW_HIGH decomposition and fp16 streams at half
    rate). Layer-1 contracts K=30 per chunk; layer-2 K=120
    block-diagonal; layer-3 accumulates k for 4 512-col units at 32-row
    offsets into one [128, 512] PSUM tile via block weight variants so
    evacuation runs at full partition width.
  - W1 and W2 are pre-halved on the host so PSUM pre-activations are a/2;
    GELU evaluates gelu(2x): ScalarE uses its exact table with the free
    input scale=2 (+ bias), VectorE uses a custom fused DVE op (single
    instruction, gelu(2x) ~= x + x^2(e0 + e1 x^2 + e2 x^4), coefficients
    minimax-fit at runtime for the provable per-layer range). The engines
    are therefore interchangeable per tile and the two GELU layers +
    k-evacuation copies are round-robined across ScalarE and VectorE.
  - h1/h2 PSUM tiles share one 3-buffer [120, 1024] ring (6 banks) so
    both GELUs run at FD=1024; emission is software-pipelined with stage
    skew A/B/C so the tensor queue never waits on a same-step GELU.
  - Device streams k back in bf16; host applies (k + b3) * f_sparse[src],
    the sorted segment mean (np.add.reduceat) and the tiny projection MLP.
"""

import numpy as np
import ml_dtypes

BF16 = ml_dtypes.bfloat16

import concourse.bass as bass
import concourse.mybir as mybir
from concourse.bacc import Bacc
from concourse.tile import TileContext
from concourse.bass_utils import run_bass_kernel_spmd

# Problem sizes (hardcoded per contract)
N_S = 131072
N_D = 131072
E = 8388608
DIM = 3
H = 12

N_CORES = 8
S = 10                      # streams (10 * 12 = 120 hidden partitions)
TW = 2048                   # cols per input tile per chunk
NCHUNK = 4                  # chunks per input tile (row bases 0/32/64/96)
NT = 13                     # input tiles per core
C_PC = NT * NCHUNK * TW     # edge-columns per core = 106496
E_PC = S * C_PC             # edges per core (padded) = 1064960
E_PAD = N_CORES * E_PC      # total padded edges = 8519680
NST = NT * NCHUNK * 2       # pipeline steps per core (1024 cols each)

# weight table columns (bf16): [0:120]=w1/2 (4 row-base variants),
# [120:240]=w2/2 (block-diag), [240:240+4*128]=w3 accumulate variants
W1C, W2C, W3C = 0, 120, 240
WCOLS = 240 + 4 * 128

_BASS_CACHE = {}
_GELU_OP = None


def _register_gelu_op():
    """Fused polynomial gelu(2x) custom DVE op (idempotent registration).
    out = u*((u*imm2 + s1)*u + s0) + in0, u = in0^2  ~=  gelu(2*in0)."""
    global _GELU_OP
    if _GELU_OP is not None:
        return _GELU_OP
    from concourse import dve_ops as dops
    from concourse.dve_spec import Spec, Src0, C0, C1, C2, sq, lower
    from concourse.dve_uop import DveOpSpec

    name = "GELU2X_POLY_ANT"
    if name in dops._SUB_OPCODE_FOR_NAME:
        _GELU_OP = next(op for op in dops.OPS if op.name == name)
        return _GELU_OP

    u = sq(Src0)
    r = (u * C2 + C1) * u + C0
    spec = Spec(
        body=u * r + Src0,
        reference=lambda in0, in1, s0, s1, imm2: (
            (in0.astype(np.float32) ** 2)
            * (((in0.astype(np.float32) ** 2) * imm2 + s1)
               * (in0.astype(np.float32) ** 2) + s0)
            + in0.astype(np.float32)
        ),
    )
    row = dops._CUSTOM_DVE_ROW_BASE + len(dops.OPS)
    shas = {}
    for ver in ("v3", "v4"):
        uops = lower(spec, ver=ver)
        shas[ver] = DveOpSpec(name=name, opcode=row, uops=uops,
                              rd1_en=False).sha(ver)
    op = dops.DveOp(name, spec, subdim=False, uops_sha=shas)
    dops.OPS.append(op)
    dops.CUSTOM_DVE_SPECS[name] = spec
    dops._SUB_OPCODE_FOR_NAME[name] = row
    _GELU_OP = op
    return op


def _fit_gelu2x_poly(xmax):
    """Fit x*erf(sqrt(2)*x) ~= u*(e0 + e1 u + e2 u^2), u=x^2, on [0, xmax],
    so that x + fit(x) == gelu(2x). Returns (coeffs, max_abs_gelu_err)."""
    from scipy.special import erf as _erf_fn
    x = np.linspace(1e-6, max(xmax, 0.125), 2001)
    u = x * x
    y = x * _erf_fn(np.sqrt(2.0) * x)
    A = np.stack([u, u * u, u * u * u], axis=1)
    w = np.ones_like(x)
    best = None
    for _ in range(120):
        c, *_ = np.linalg.lstsq(A * w[:, None], (y * w)[:, None], rcond=None)
        c = c[:, 0]
        err = A @ c - y
        m = np.abs(err).max()
        if best is None or m < best[1]:
            best = (c, m)
        w = w * (0.9 + 0.25 * np.abs(err) / m)
        w /= w.max()
    return best


def _plan(W1, b1, W2, b2):
    """Per-layer DVE gelu(2x) coefficients for the halved-weight scheme.
    Requires b1 == b2 == 0 for DVE tiles; None -> exact ScalarE only."""
    if np.any(np.asarray(b1) != 0) or np.any(np.asarray(b2) != 0):
        return None
    W1 = np.asarray(W1, np.float64)
    W2 = np.asarray(W2, np.float64)
    r1 = 0.5 * np.abs(W1).sum(axis=0)            # per-hidden |a1| bound
    h1max = np.maximum(_gelu_np(r1), 0.17)       # |gelu| <= max(gelu(r), .17)
    r2 = np.abs(W2).T @ h1max                    # per-hidden |a2| bound
    x1 = float(r1.max()) * 0.51 + 0.01           # domain of a1/2
    x2 = float(r2.max()) * 0.51 + 0.01           # domain of a2/2
    c1, e1m = _fit_gelu2x_poly(x1)
    c2, e2m = _fit_gelu2x_poly(x2)
    if e1m > 2e-3 or e2m > 8e-3:
        return None
    rd = lambda c: tuple(round(float(v), 10) for v in c)
    return rd(c1), rd(c2)


def _build_bass(plan):
    key = plan
    if key in _BASS_CACHE:
        return _BASS_CACHE[key]
    fp32 = mybir.dt.float32
    bf16 = mybir.dt.bfloat16
    GELU = mybir.ActivationFunctionType.Gelu
    use_dve = plan is not None
    if use_dve:
        gop = _register_gelu_op()
        c1, c2 = plan
    else:
        c1 = c2 = None

    nc = Bacc()
    xin = nc.dram_tensor("xin", [NT, 128, TW], bf16, kind="ExternalInput")
    wtab = nc.dram_tensor("wtab", [128, WCOLS], bf16, kind="ExternalInput")
    btab = nc.dram_tensor("btab", [128, 2], fp32, kind="ExternalInput")
    kout = nc.dram_tensor("kout", [NT, 128, TW], bf16, kind="ExternalOutput")

    with TileContext(nc) as tc:
        with (
            tc.tile_pool(name="wpool", bufs=1) as wpool,
            tc.tile_pool(name="inpool", bufs=3) as inpool,
            tc.tile_pool(name="h1gpool", bufs=3) as h1gpool,
            tc.tile_pool(name="h2gpool", bufs=4) as h2gpool,
            tc.tile_pool(name="kspool", bufs=2) as kspool,
            tc.tile_pool(name="pp", bufs=3, space="PSUM") as pp,
            tc.tile_pool(name="pk", bufs=2, space="PSUM") as pk,
        ):
            wt = wpool.tile([128, WCOLS], bf16, tag="wt")
            nc.sync.dma_start(wt[:], wtab[:, :])
            bt = wpool.tile([128, 2], fp32, tag="bt")
            nc.sync.dma_start(bt[:], btab[:, :])
            w1v = [wt[32 * c:32 * c + 30, W1C:W1C + 120] for c in range(NCHUNK)]
            w2s = wt[0:120, W2C:W2C + 120]
            w3v = [wt[0:120, W3C + 128 * m:W3C + 128 * (m + 1)]
                   for m in range(4)]
            b2t = bt[0:120, 0:1]
            b1t = bt[0:120, 1:2]

            xts = [None] * NT
            h1gs = [None] * NST
            h2gs = [None] * NST
            kas = {}
            kss = [None] * NT

            def gelu_emit(dve, coefs, bias_ap, out_ap, in_ap):
                if dve and use_dve:
                    e0, e1, e2 = coefs
                    nc.vector._custom_dve(gop, out=out_ap, in0=in_ap,
                                          s0=float(e0), s1=float(e1),
                                          imm2=float(e2))
                else:
                    nc.scalar.activation(out_ap, in_ap, GELU,
                                         bias=bias_ap, scale=2.0)

            def stage_a(i):
                t, r = divmod(i, NCHUNK * 2)
                c, o = divmod(r, 2)
                if r == 0:
                    xt = inpool.tile([128, TW], bf16, tag="x")
                    nc.sync.dma_start(xt[:], xin[t, :, :])
                    xts[t] = xt
                xt = xts[t]
                h1 = pp.tile([120, 1024], fp32, tag="hp")
                for q in range(2):
                    col = 1024 * o + 512 * q
                    nc.tensor.matmul(
                        h1[:, 512 * q:512 * q + 512], w1v[c],
                        xt[32 * c:32 * c + 30, col:col + 512],
                        start=True, stop=True, tile_position=(32 * c, 0))
                h1g = h1gpool.tile([120, 1024], bf16, tag="h1g")
                gelu_emit(i % 2 == 0, c1, b1t, h1g[:], h1[:])
                h1gs[i] = h1g

            def stage_b(i):
                h1g = h1gs[i]
                h2 = pp.tile([120, 1024], fp32, tag="hp")
                for q in range(2):
                    nc.tensor.matmul(h2[:, 512 * q:512 * q + 512], w2s,
                                     h1g[:, 512 * q:512 * q + 512],
                                     start=True, stop=True)
                h1gs[i] = None
                h2g = h2gpool.tile([120, 1024], bf16, tag="h2g")
                gelu_emit(i % 2 == 1, c2, b2t, h2g[:], h2[:])
                h2gs[i] = h2g

            def stage_c(i):
                t, r = divmod(i, NCHUNK * 2)
                c, o = divmod(r, 2)
                if o == 0:
                    kas[i // 2] = pk.tile([128, 512], fp32, tag="ka",
                                          name="ka")
                ka = kas[i // 2]
                for q in range(2):
                    m = 2 * o + q
                    nc.tensor.matmul(ka[:], w3v[m],
                                     h2gs[i][:, 512 * q:512 * q + 512],
                                     start=(m == 0), stop=(m == 3))
                h2gs[i] = None
                if o == 1:
                    if c == 0:
                        ks = kspool.tile([128, TW], bf16, tag="ks")
                        kss[t] = ks
                    ks = kss[t]
                    if c % 2 == 0:
                        nc.scalar.copy(ks[:, 512 * c:512 * c + 512], ka[:])
                    else:
                        nc.vector.tensor_copy(ks[:, 512 * c:512 * c + 512],
                                              ka[:])
                    kas[i // 2] = None
                    if c == NCHUNK - 1:
                        nc.gpsimd.dma_start(kout[t, :, :], ks[:])
                        kss[t] = None

            for i in range(NST + 2):
                if i < NST:
                    stage_a(i)
                if 1 <= i < NST + 1:
                    stage_b(i - 1)
                if 2 <= i < NST + 2:
                    stage_c(i - 2)

    nc.finalize()
    _BASS_CACHE[key] = nc
    return nc


def _erf(x):
    # Abramowitz & Stegun 7.1.26 fallback (|err| <= 1.5e-7)
    a1, a2, a3, a4, a5 = (0.254829592, -0.284496736, 1.421413741,
                          -1.453152027, 1.061405429)
    p = 0.3275911
    s = np.sign(x)
    ax = np.abs(x)
    t = 1.0 / (1.0 + p * ax)
    y = 1.0 - (((((a5 * t + a4) * t) + a3) * t + a2) * t + a1) * t * np.exp(-ax * ax)
    return s * y

try:
    from scipy.special import erf as _erf  # noqa: F811
except Exception:
    pass


def _gelu_np(x):
    return 0.5 * x * (1.0 + _erf(x / np.sqrt(2.0)))


def _pack_inputs(x_sparse, f_sparse, x_dense, W1, b1, W2, b2, W3, b3,
                 edge_src, edge_dst, plan):
    src = np.asarray(edge_src).astype(np.int64)
    dst = np.asarray(edge_dst).astype(np.int64)
    x_sparse = np.asarray(x_sparse, dtype=np.float32)
    x_dense = np.asarray(x_dense, dtype=np.float32)

    rel = np.zeros((E_PAD, DIM), BF16)
    rel[:E] = (x_sparse[src] - x_dense[dst]).astype(BF16)

    # W1/W2 halved: PSUM pre-activations are a/2; GELU evaluates gelu(2x).
    W1 = np.asarray(W1, np.float32) * 0.5
    W2 = np.asarray(W2, np.float32) * 0.5
    W3 = np.asarray(W3, np.float32)

    wtab = np.zeros((128, WCOLS), BF16)
    rs = np.arange(S)
    for c in range(NCHUNK):
        for j in range(DIM):
            wtab[(32 * c + 3 * rs + j)[:, None],
                 W1C + 12 * rs[:, None] + np.arange(H)] = W1[j].astype(BF16)
    for i in range(H):
        wtab[(12 * rs + i)[:, None], W2C + 12 * rs[:, None] + np.arange(H)] \
            = W2[i].astype(BF16)
    for m in range(4):
        for i in range(H):
            wtab[(12 * rs + i)[:, None], W3C + 128 * m + 32 * m
                 + 3 * rs[:, None] + np.arange(DIM)] = W3[i].astype(BF16)
    btab = np.zeros((128, 2), np.float32)
    btab[12 * rs[:, None] + np.arange(H), 0] = np.asarray(b2, np.float32)
    btab[12 * rs[:, None] + np.arange(H), 1] = np.asarray(b1, np.float32)

    in_maps = []
    for cr in range(N_CORES):
        relc = rel[cr * E_PC:(cr + 1) * E_PC]
        # [S, NT, NCHUNK, TW, DIM] -> [NT, NCHUNK, S, DIM, TW]
        x5 = relc.reshape(S, NT, NCHUNK, TW, DIM).transpose(1, 2, 0, 4, 3)
        x4 = np.zeros((NT, NCHUNK, 32, TW), BF16)
        x4[:, :, :30, :] = x5.reshape(NT, NCHUNK, 30, TW)
        in_maps.append({
            "xin": x4.reshape(NT, 128, TW),
            "wtab": wtab,
            "btab": btab,
        })
    return in_maps, src, dst


def _host_tail(outs, src, dst, f_sparse, b3, P1w, P1b, P2w, P2b, P3w, P3b):
    f_sparse = np.asarray(f_sparse, np.float32)
    b3 = np.asarray(b3, np.float32)
    k = np.empty((E_PAD, DIM), np.float32)
    for cr in range(N_CORES):
        ko = np.asarray(outs[cr]["kout"])  # [NT, 128, TW] bf16
        # rows: 32*(2o+q) + 3s + j ; cols: 512*c + v
        # edge col within core = t*8192 + c*2048 + o*1024 + q*512 + v
        k6 = ko.reshape(NT, 4, 32, NCHUNK, 512)[:, :, :30, :, :]
        k6 = k6.reshape(NT, 4, S, DIM, NCHUNK, 512)
        # axes [t, m(=2o+q), s, j, c, v] -> [s, t, c, m, v, j]
        k6 = k6.transpose(2, 0, 4, 1, 5, 3)
        k[cr * E_PC:(cr + 1) * E_PC] = k6.reshape(E_PC, DIM).astype(np.float32)
    k = k[:E]

    msg = (k + b3) * f_sparse[src]

    cnt = np.bincount(dst, minlength=N_D).astype(np.float32)
    starts = (np.cumsum(cnt) - cnt).astype(np.int64)
    nz = cnt > 0
    sums = np.zeros((N_D, DIM), np.float32)
    if nz.any():
        sums[nz] = np.add.reduceat(msg, starts[nz], axis=0)
    out_feat = sums / np.maximum(cnt, 1.0)[:, None]

    h = _gelu_np(out_feat.astype(np.float64) @ np.asarray(P1w, np.float64)
                 + np.asarray(P1b, np.float64))
    h = _gelu_np(h @ np.asarray(P2w, np.float64) + np.asarray(P2b, np.float64))
    out = h @ np.asarray(P3w, np.float64) + np.asarray(P3b, np.float64)
    return out.astype(np.float32)


def kernel(x_sparse, f_sparse, x_dense, W1, b1, W2, b2, W3, b3,
           P1w, P1b, P2w, P2b, P3w, P3b, edge_src, edge_dst):
    plan = _plan(W1, b1, W2, b2)
    in_maps, src, dst = _pack_inputs(x_sparse, f_sparse, x_dense, W1, b1,
                                     W2, b2, W3, b3, edge_src, edge_dst,
                                     plan)
    nc = _build_bass(plan)
    res = run_bass_kernel_spmd(nc, in_maps, list(range(N_CORES)))
    return _host_tail(res.results, src, dst, f_sparse, b3,
                      P1w, P1b, P2w, P2b, P3w, P3b)


def run_profiled(inputs, tmpdir=None):
    """Run once with tracing enabled; returns BassKernelResults."""
    kw = {k: inputs[k] for k in ("x_sparse", "f_sparse", "x_dense", "W1",
                                 "b1", "W2", "b2", "W3", "b3",
                                 "edge_src", "edge_dst")}
    plan = _plan(kw["W1"], kw["b1"], kw["W2"], kw["b2"])
    in_maps, _, _ = _pack_inputs(**kw, plan=plan)
    nc = _build_bass(plan)
    return run_bass_kernel_spmd(nc, in_maps, list(range(N_CORES)),
                                trace=True, tmpdir=tmpdir)
